# revision 10
# baseline (speedup 1.0000x reference)
"""Trainium2 Bass kernel for a dense causal-attention transformer block.

Computes: qkv projections + RoPE + causal softmax attention + output
projection, matching the reference jax implementation with
B=2, S=2048, D=2048, 16 heads x 128 head-dim, on 8 NeuronCores.

Sharding: data-parallel over batch (2 groups of 4 cores) x tensor-parallel
over heads (4 heads per core). Attention is fully head-local; the only
communication is an AllGather of the per-core attention outputs (bf16)
within each 4-core batch group before the wo matmul, of which each core
computes a 512-wide output-column slice.

v2 scheduling: host-relayout batched DMAs (one submission per weight /
x column-chunk), head-0 attention woven into the prologue rounds,
next-head projection matmuls woven into attention chunks as PE filler
(exp on the scalar engine paces the attention inner loop), softmax
denominators tree-accumulated on the DVE with one ones-matmul per
chunk, bf16 RoPE intermediates.
"""
import os
import sys
import types

sys.path.insert(0, "/opt/trn_rl_repo")

import numpy as np


def _install_ntff_hook():
    """Recreate the missing antenv.axon_hooks module so trace=True works."""
    try:
        import antenv

        if "antenv.axon_hooks" in sys.modules:
            return
        m = types.ModuleType("antenv.axon_hooks")
        m._hook = None

        def set_axon_ntff_profile_hook(h):
            m._hook = h

        def get_axon_ntff_profile_hook():
            return m._hook

        m.set_axon_ntff_profile_hook = set_axon_ntff_profile_hook
        m.get_axon_ntff_profile_hook = get_axon_ntff_profile_hook
        sys.modules["antenv.axon_hooks"] = m
        antenv.axon_hooks = m
        from trn_agent_boot.trn_boot import _ntff_profile_via_ctypes

        so = "/opt/axon/libaxon_pjrt.so"
        if os.path.exists(so):
            set_axon_ntff_profile_hook(_ntff_profile_via_ctypes(so))
    except Exception:
        pass


_install_ntff_hook()

import ml_dtypes
import concourse.bass as bass
import concourse.tile as tile
from concourse import bacc, mybir
from concourse.bass_utils import run_bass_kernel_spmd

BF16 = mybir.dt.bfloat16
F32 = mybir.dt.float32

B, S, D = 2, 2048, 2048
QH, H = 16, 128          # heads, head dim
N_CORES = 8
GROUPS = 4               # tensor-parallel groups per batch
HPC = QH // GROUPS       # heads per core = 4
DQC = HPC * H            # per-core projection width = 512
NT = S // 128            # 16 s/d tiles of 128
NC = S // 512            # 4 chunks of 512
KT = D // 128            # 16 contraction tiles

LAST_RESULTS = None      # test harness reads exec_time_ns from here


def _build():
    nc = bacc.Bacc("TRN2", target_bir_lowering=False, debug=False,
                   num_devices=N_CORES)

    xt = [nc.dram_tensor(f"xt{sc}", [128, KT, 512], BF16,
                         kind="ExternalInput") for sc in range(NC)]
    wq = nc.dram_tensor("wq", [128, KT, DQC], BF16, kind="ExternalInput")
    wk = nc.dram_tensor("wk", [128, KT, DQC], BF16, kind="ExternalInput")
    wv = nc.dram_tensor("wv", [128, KT, DQC], BF16, kind="ExternalInput")
    wo = nc.dram_tensor("wo", [128, KT, DQC], BF16, kind="ExternalInput")
    cosT = nc.dram_tensor("cosT", [H, S], BF16, kind="ExternalInput")
    sinTs = nc.dram_tensor("sinTs", [H, S], BF16, kind="ExternalInput")
    tri = nc.dram_tensor("tri", [128, 128], BF16, kind="ExternalInput")
    out = nc.dram_tensor("out", [S, DQC], F32, kind="ExternalOutput")

    with tile.TileContext(nc) as tc:
        with (
            tc.tile_pool(name="big", bufs=4) as big_pool,      # xt / ytf pieces
            tc.tile_pool(name="wts", bufs=1) as wts_pool,      # weights
            tc.tile_pool(name="qkv", bufs=1) as qkv_pool,      # qt/kt/v/subs
            tc.tile_pool(name="small", bufs=1) as small_pool,  # constants
            tc.tile_pool(name="work", bufs=3) as work_pool,    # rope/at/tmp
            tc.tile_pool(name="psum", bufs=2, space="PSUM") as psum_pool,
            tc.tile_pool(name="dram", bufs=1, space="DRAM") as dram_pool,
        ):
            # ---- batched input DMAs, ordered for earliest PE start --------
            # inputs stream on two DMA submission queues in parallel:
            # sync carries wq/xt0/wv/xt2, scalar carries wk/xt1/xt3
            wq_sb = wts_pool.tile([128, KT, DQC], BF16, tag="wq", name="wq")
            nc.sync.dma_start(wq_sb[:], wq[:])
            xt_sb = []
            for sc in range(NC):
                t = big_pool.tile([128, KT, 512], BF16, tag="big",
                                  name=f"xt{sc}")
                xt_sb.append(t)
            wk_sb = wts_pool.tile([128, KT, DQC], BF16, tag="wk", name="wk")
            nc.scalar.dma_start(wk_sb[:], wk[:])
            nc.sync.dma_start(xt_sb[0][:], xt[0][:])
            nc.scalar.dma_start(xt_sb[1][:], xt[1][:])
            wv_sb = wts_pool.tile([128, KT, DQC], BF16, tag="wvo", name="wv")
            nc.sync.dma_start(wv_sb[:], wv[:])
            nc.scalar.dma_start(xt_sb[3][:], xt[3][:])
            nc.sync.dma_start(xt_sb[2][:], xt[2][:])
            cos_sb = small_pool.tile([H, S], BF16, tag="cos", name="cos")
            nc.sync.dma_start(cos_sb[:], cosT[:])
            sin_sb = small_pool.tile([H, S], BF16, tag="sin", name="sin")
            nc.sync.dma_start(sin_sb[:], sinTs[:])
            tri_sb = small_pool.tile([128, 128], BF16, tag="tri", name="tri")
            nc.sync.dma_start(tri_sb[:], tri[:])
            ones_sb = small_pool.tile([128, 1], BF16, tag="ones", name="ones")
            nc.vector.memset(ones_sb[:], 1.0)
            onesr_sb = small_pool.tile([1, 128], BF16, tag="onesr",
                                       name="onesr")
            nc.vector.memset(onesr_sb[:], 1.0)

            qt_sb = [qkv_pool.tile([H, S], BF16, tag=f"qt{m}", name=f"qt{m}")
                     for m in range(HPC)]
            kt_sb = [qkv_pool.tile([H, S], BF16, tag=f"kt{m}", name=f"kt{m}")
                     for m in range(HPC)]
            v_sb = [qkv_pool.tile([128, DQC], BF16, tag=f"v{i}", name=f"v{i}")
                    for i in range(NT)]

            rope_n = [0]

            def rope(ps, dst, sc):
                """dst[:, sc-chunk] = ps*cos + swap_halves(ps)*sin_signed."""
                scol = slice(sc * 512, (sc + 1) * 512)
                i = rope_n[0]
                rope_n[0] += 1
                t_rot = work_pool.tile([128, 512], BF16, tag="t_rot",
                                       bufs=2, name=f"t_rot{i}")
                nc.vector.tensor_copy(t_rot[0:64, :], ps[64:128, :])
                nc.vector.tensor_copy(t_rot[64:128, :], ps[0:64, :])
                t_cos = work_pool.tile([128, 512], BF16, tag="t_cos",
                                       bufs=2, name=f"t_cos{i}")
                nc.vector.tensor_tensor(t_cos[:], ps[:], cos_sb[:, scol],
                                        mybir.AluOpType.mult)
                t_sin = work_pool.tile([128, 512], BF16, tag="t_sin",
                                       bufs=2, name=f"t_sin{i}")
                nc.vector.tensor_tensor(t_sin[:], t_rot[:], sin_sb[:, scol],
                                        mybir.AluOpType.mult)
                nc.vector.tensor_tensor(dst[:, scol], t_cos[:], t_sin[:],
                                        mybir.AluOpType.add)

            def proj_chain(m, w_sb, dst, sc, nm):
                """One 16-matmul projection chain + RoPE for (head m, chunk sc)."""
                ps = psum_pool.tile([128, 512], F32, tag="pp", bufs=2,
                                    name=f"pp_{nm}{m}{sc}")
                for kd in range(KT):
                    nc.tensor.matmul(
                        ps[:],
                        w_sb[:, kd, m * 128:(m + 1) * 128],
                        xt_sb[sc][:, kd, :],
                        start=(kd == 0), stop=(kd == KT - 1),
                    )
                rope(ps, dst, sc)

            def proj_thunks(m):
                """Per-matmul thunks for head m's 8 proj chains; each thunk
                returns True iff it emitted a PE matmul."""
                thunks = []
                for (w_sb, dst, nm) in ((wq_sb, qt_sb[m], "q"),
                                        (wk_sb, kt_sb[m], "k")):
                    for sc in range(NC):
                        hold = [None]

                        def mk_mm(kd, w_sb=w_sb, sc=sc, hold=hold, m=m, nm=nm):
                            def f():
                                if kd == 0:
                                    hold[0] = psum_pool.tile(
                                        [128, 512], F32, tag="pp", bufs=2,
                                        name=f"pp_{nm}{m}{sc}")
                                nc.tensor.matmul(
                                    hold[0][:],
                                    w_sb[:, kd, m * 128:(m + 1) * 128],
                                    xt_sb[sc][:, kd, :],
                                    start=(kd == 0), stop=(kd == KT - 1),
                                )
                                return True
                            return f

                        for kd in range(KT):
                            thunks.append(mk_mm(kd))

                        def mk_rope(dst=dst, sc=sc, hold=hold):
                            def f():
                                rope(hold[0], dst, sc)
                                return False
                            return f

                        thunks.append(mk_rope())
                return thunks

            def pop_filler(fillers):
                """Emit filler thunks until one PE matmul lands (or drained)."""
                while True:
                    th = next(fillers, None)
                    if th is None or th():
                        return

            def v_chain(i):
                """V projection for s-tile i."""
                sc, o = i // 4, (i % 4) * 128
                ps = psum_pool.tile([128, DQC], F32, tag="pp", bufs=2,
                                    name=f"pp_v{i}")
                for kd in range(KT):
                    nc.tensor.matmul(
                        ps[:],
                        xt_sb[sc][:, kd, o:o + 128],
                        wv_sb[:, kd, :],
                        start=(kd == 0), stop=(kd == KT - 1),
                    )
                nc.scalar.copy(v_sb[i][:], ps[:])

            # dram staging: one piece AllGather per head 0-2 (collectives
            # cost ~25us each regardless of size, so fewer is better), plus
            # per-q-chunk sub-gathers for head 3 that pipeline into stage F
            yt_piece = [dram_pool.tile([128, S], BF16, name=f"yt_p{m}")
                        for m in range(HPC - 1)]
            ytf_piece = [dram_pool.tile([DQC, S], BF16, name=f"ytf_p{m}")
                         for m in range(HPC - 1)]
            yt3_sub = [dram_pool.tile([128, 512], BF16, name=f"yt3_s{j}")
                       for j in range(NC)]
            ytf3_sub = [dram_pool.tile([DQC, 512], BF16, name=f"ytf3_s{j}")
                        for j in range(NC)]
            ytf_sb = [None] * (HPC - 1)   # heads 0-2 land in dead xt tiles
            ytf3sub_sb = [[None] * GROUPS for _ in range(NC)]

            # warmup collective: absorbs the expensive first-op setup (~40-
            # 100us observed) while the PE chews the prologue
            warm_in = dram_pool.tile([1, 64], BF16, name="warm_in")
            warm_out = dram_pool.tile([GROUPS, 64], BF16, name="warm_out")
            warm_sb = small_pool.tile([1, 64], BF16, tag="warm", name="warm")
            nc.vector.memset(warm_sb[:], 0.0)
            nc.sync.dma_start(warm_in[:], warm_sb[:])
            nc.gpsimd.collective_compute(
                "AllGather",
                mybir.AluOpType.bypass,
                replica_groups=[[0, 1, 2, 3], [4, 5, 6, 7]],
                ins=[warm_in.opt()],
                outs=[warm_out.opt()],
            )

            def load_piece(mh):
                """Load head mh's gathered piece into a dead xt tile."""
                t = big_pool.tile([128, KT, 512], BF16, tag="big",
                                  name=f"ytf{mh}")
                for p in range(GROUPS):
                    nc.sync.dma_start(
                        t[:, 4 * p:4 * p + 4, :],
                        ytf_piece[mh][p * 128:(p + 1) * 128, :])
                ytf_sb[mh] = t

            att_n = [0]

            def attn_head(m, chunks, fillers):
                """Causal attention for head m over the given q-chunks."""
                for j in chunks:
                    n_kv = 4 * j + 4
                    u = att_n[0]
                    att_n[0] += 1
                    ps_y = psum_pool.tile([128, 512], F32, tag="ps_y",
                                          bufs=2, name=f"ps_y{u}")
                    pden = psum_pool.tile([128, 512], F32, tag="pden",
                                          bufs=1, name=f"pden{u}")
                    state = {"pend": None, "accD": None, "pacc": []}

                    def kv_scores(t, u=u, m=m, j=j):
                        c0 = max(0, (t - 4 * j) * 128)
                        ps_s = psum_pool.tile([128, 512], F32, tag="ps_s",
                                              bufs=3, name=f"ps_s{u}_{t}")
                        nc.tensor.matmul(
                            ps_s[:, c0:512],
                            kt_sb[m][:, t * 128:(t + 1) * 128],
                            qt_sb[m][:, j * 512 + c0:(j + 1) * 512],
                            start=True, stop=True,
                        )
                        at = work_pool.tile([128, 512], BF16, tag="at",
                                            bufs=4, name=f"at{u}_{t}")
                        nc.scalar.activation(
                            at[:, c0:512], ps_s[:, c0:512],
                            mybir.ActivationFunctionType.Exp)
                        if t >= 4 * j:
                            nc.vector.tensor_tensor(
                                at[:, c0:c0 + 128], at[:, c0:c0 + 128],
                                tri_sb[:], mybir.AluOpType.mult)
                        return at, c0

                    def kv_accum(at, c0, t, u=u, m=m, j=j, n_kv=n_kv,
                                 ps_y=ps_y, state=state, fillers=fillers):
                        nc.tensor.matmul(
                            ps_y[:, c0:512],
                            v_sb[t][:, m * 128:(m + 1) * 128],
                            at[:, c0:512],
                            start=(t == 0), stop=(t == n_kv - 1),
                        )
                        pop_filler(fillers)
                        # denominator accumulation on the DVE
                        if t < 4 * j:
                            if state["pend"] is None:
                                state["pend"] = at
                            else:
                                p = work_pool.tile([128, 512], BF16,
                                                   tag="pacc", bufs=6,
                                                   name=f"pacc{u}_{t}")
                                nc.vector.tensor_tensor(
                                    p[:], state["pend"][:], at[:],
                                    mybir.AluOpType.add)
                                state["pacc"].append(p)
                                state["pend"] = None
                        else:
                            if state["accD"] is None:
                                state["accD"] = at       # t == 4j: c0 == 0
                            else:
                                nc.vector.tensor_tensor(
                                    state["accD"][:, c0:512],
                                    state["accD"][:, c0:512],
                                    at[:, c0:512], mybir.AluOpType.add)

                    # software pipeline: scores(t+1) issues before AV(t)
                    prev_at, prev_c0 = kv_scores(0)
                    for t in range(1, n_kv):
                        cur = kv_scores(t)
                        kv_accum(prev_at, prev_c0, t - 1)
                        prev_at, prev_c0 = cur
                    kv_accum(prev_at, prev_c0, n_kv - 1)

                    accD = state["accD"]
                    for p in state["pacc"]:
                        nc.vector.tensor_tensor(accD[:], accD[:], p[:],
                                                mybir.AluOpType.add)
                    # denominator row, reciprocal, then broadcast it back
                    # across partitions with a K=1 matmul (keeps the gpsimd
                    # queue free for collective dispatches)
                    nc.tensor.matmul(pden[0:1, :], ones_sb[:], accD[:],
                                     start=True, stop=True)
                    d_sb = work_pool.tile([1, 512], F32, tag="d_sb", bufs=2,
                                          name=f"d_sb{u}")
                    d_scr = work_pool.tile([1, 512], F32, tag="d_scr", bufs=2,
                                           name=f"d_scr{u}")
                    nc.vector.reciprocal_approx_accurate(
                        d_sb[:], pden[0:1, :], d_scr[:])
                    d_bf = work_pool.tile([1, 512], BF16, tag="d_bf", bufs=2,
                                          name=f"d_bf{u}")
                    nc.vector.tensor_copy(d_bf[:], d_sb[:])
                    nc.tensor.matmul(pden[:, :], onesr_sb[:], d_bf[:],
                                     start=True, stop=True)
                    b_sb = work_pool.tile([128, 512], BF16, tag="b_sb",
                                          bufs=2, name=f"b_sb{u}")
                    nc.vector.tensor_copy(b_sb[:], pden[:, :])
                    ytile = work_pool.tile([128, 512], BF16, tag="ytile",
                                           bufs=3, name=f"ytile{u}")
                    nc.vector.tensor_tensor(
                        ytile[:], ps_y[:], b_sb[:], mybir.AluOpType.mult)
                    if m == HPC - 1:
                        nc.sync.dma_start(yt3_sub[j][:], ytile[:])
                        nc.gpsimd.collective_compute(
                            "AllGather",
                            mybir.AluOpType.bypass,
                            replica_groups=[[0, 1, 2, 3], [4, 5, 6, 7]],
                            ins=[yt3_sub[j].opt()],
                            outs=[ytf3_sub[j].opt()],
                        )
                        if j < HPC - 2:
                            load_piece(j)
                    else:
                        nc.sync.dma_start(
                            yt_piece[m][:, j * 512:(j + 1) * 512], ytile[:])

            NO_FILL = iter(())

            # ---- prologue rounds, ordered to match DMA arrivals -----------
            proj_chain(0, wq_sb, qt_sb[0], 0, "q")
            proj_chain(0, wk_sb, kt_sb[0], 0, "k")
            proj_chain(0, wq_sb, qt_sb[0], 1, "q")
            proj_chain(0, wk_sb, kt_sb[0], 1, "k")
            for i in range(4):
                v_chain(i)
            attn_head(0, [0], NO_FILL)
            for i in range(4, 8):
                v_chain(i)
            proj_chain(0, wq_sb, qt_sb[0], 2, "q")
            proj_chain(0, wk_sb, kt_sb[0], 2, "k")
            for i in range(8, 12):
                v_chain(i)
            attn_head(0, [1], NO_FILL)
            proj_chain(0, wq_sb, qt_sb[0], 3, "q")
            proj_chain(0, wk_sb, kt_sb[0], 3, "k")
            for i in range(12, 16):
                v_chain(i)
            attn_head(0, [2], NO_FILL)
            # wo reuses wv's SBUF slot once the V chains drain
            wo_sb = wts_pool.tile([128, KT, DQC], BF16, tag="wvo", name="wo")
            nc.sync.dma_start(wo_sb[:], wo[:])

            # ---- attention train with woven next-head projections ---------
            def ag_piece(m):
                nc.gpsimd.collective_compute(
                    "AllGather",
                    mybir.AluOpType.bypass,
                    replica_groups=[[0, 1, 2, 3], [4, 5, 6, 7]],
                    ins=[yt_piece[m].opt()],
                    outs=[ytf_piece[m].opt()],
                )

            fillers = iter(proj_thunks(1))
            attn_head(0, [NC - 1], fillers)
            ag_piece(0)
            for th in fillers:
                th()
            for m in range(1, HPC):
                fillers = iter(proj_thunks(m + 1)) if m < HPC - 1 else NO_FILL
                attn_head(m, list(range(NC)), fillers)
                if m < HPC - 1:
                    ag_piece(m)
                for th in fillers:
                    th()

            load_piece(2)
            # head-3 gathered sub-pieces land in dead v slots
            for j in range(NC):
                for p in range(GROUPS):
                    t = qkv_pool.tile([128, 512], BF16, tag=f"v{4 * j + p}",
                                      name=f"y3s{j}_{p}")
                    nc.sync.dma_start(
                        t[:], ytf3_sub[j][p * 128:(p + 1) * 128, :])
                    ytf3sub_sb[j][p] = t

            # ---- stage F: output projection (512-col slice) ---------------
            for ms in range(NT):
                ps = psum_pool.tile([128, DQC], F32, tag="pp", bufs=2,
                                    name=f"pp_o{ms}")
                for mh in range(HPC):
                    for p in range(GROUPS):
                        if mh == HPC - 1:
                            lhsT = ytf3sub_sb[ms // 4][p][
                                :, (ms % 4) * 128:(ms % 4 + 1) * 128]
                        else:
                            lhsT = ytf_sb[mh][
                                :, 4 * p + ms // 4,
                                (ms % 4) * 128:(ms % 4) * 128 + 128]
                        nc.tensor.matmul(
                            ps[:],
                            lhsT,
                            wo_sb[:, 4 * p + mh, :],
                            start=(mh == 0 and p == 0),
                            stop=(mh == HPC - 1 and p == GROUPS - 1),
                        )
                o_sb = work_pool.tile([128, DQC], F32, tag="o_sb", bufs=2,
                                      name=f"o_sb{ms}")
                nc.scalar.copy(o_sb[:], ps[:])
                nc.sync.dma_start(out[ms * 128:(ms + 1) * 128, :], o_sb[:])

    nc.compile()
    return nc


_NC_CACHE = None


def kernel(x, wq, wk, wv, wo, mask, sin, cos):
    global LAST_RESULTS, _NC_CACHE
    bf16 = ml_dtypes.bfloat16

    def w_relayout(w):
        # [D, DQC] -> [128, KT, DQC]: partition p holds rows {kd*128+p}
        return np.ascontiguousarray(
            w.reshape(KT, 128, -1).transpose(1, 0, 2)).astype(bf16)

    wq_b = wq.astype(np.float32)
    wk_b = wk.astype(np.float32) * (H ** -0.5)   # fold k scaling into wk
    wv_b = wv.astype(np.float32)
    wo_b = wo.astype(np.float32)

    # transposed rope tables; sign-folded sin handles rotate_half:
    #   q'[0:64]   = q[0:64]*cos[0:64]   + q[64:128]*(-sin[0:64])
    #   q'[64:128] = q[64:128]*cos[64:]  + q[0:64]  *(+sin[64:])
    cosT = np.ascontiguousarray(cos.T).astype(bf16)                  # [H, S]
    sinT = np.ascontiguousarray(sin.T).astype(np.float32)
    sinTs = sinT.copy()
    sinTs[0:H // 2, :] *= -1.0
    sinTs = sinTs.astype(bf16)

    # multiplicative causal mask for the 128x128 diagonal blocks, in
    # [kv, q] orientation, derived from the additive mask input
    tri = (mask[:128, :128].T == 0.0).astype(bf16)

    if _NC_CACHE is None:
        _NC_CACHE = _build()
    nc = _NC_CACHE

    in_maps = []
    for c in range(N_CORES):
        b, g = c // GROUPS, c % GROUPS
        cols = slice(g * DQC, (g + 1) * DQC)
        # x[b].T -> [D, S] -> [NC, 128, KT, 512] column chunks
        xt = np.ascontiguousarray(x[b].T.astype(np.float32))
        xt4 = np.ascontiguousarray(
            xt.reshape(KT, 128, NC, 512).transpose(2, 1, 0, 3)).astype(bf16)
        in_maps.append({
            "xt0": np.ascontiguousarray(xt4[0]),
            "xt1": np.ascontiguousarray(xt4[1]),
            "xt2": np.ascontiguousarray(xt4[2]),
            "xt3": np.ascontiguousarray(xt4[3]),
            "wq": w_relayout(wq_b[:, cols]),
            "wk": w_relayout(wk_b[:, cols]),
            "wv": w_relayout(wv_b[:, cols]),
            "wo": w_relayout(wo_b[:, cols]),
            "cosT": cosT,
            "sinTs": sinTs,
            "tri": tri,
        })

    try:
        res = run_bass_kernel_spmd(nc, in_maps, core_ids=list(range(N_CORES)))
    except Exception:
        # transient device states (e.g. a prior crashed load) sometimes
        # surface as unrecoverable-execution errors; one retry clears them
        import time

        time.sleep(5)
        res = run_bass_kernel_spmd(nc, in_maps, core_ids=list(range(N_CORES)))
    LAST_RESULTS = res

    output = np.empty((B, S, D), dtype=np.float32)
    for c in range(N_CORES):
        b, g = c // GROUPS, c % GROUPS
        output[b, :, g * DQC:(g + 1) * DQC] = res.results[c]["out"]
    return output


# revision 11
# speedup vs baseline: 1.0585x; 1.0585x over previous
"""Trainium2 Bass kernel for a dense causal-attention transformer block.

Computes: qkv projections + RoPE + causal softmax attention + output
projection, matching the reference jax implementation with
B=2, S=2048, D=2048, 16 heads x 128 head-dim, on 8 NeuronCores.

Sharding: data-parallel over batch (2 groups of 4 cores) x tensor-parallel
over heads (4 heads per core). Attention is fully head-local; the only
communication is an AllGather of the per-core attention outputs (bf16)
within each 4-core batch group before the wo matmul, of which each core
computes a 512-wide output-column slice.

v2 scheduling: host-relayout batched DMAs (one submission per weight /
x column-chunk), head-0 attention woven into the prologue rounds,
next-head projection matmuls woven into attention chunks as PE filler
(exp on the scalar engine paces the attention inner loop), softmax
denominators tree-accumulated on the DVE with one ones-matmul per
chunk, bf16 RoPE intermediates.
"""
import os
import sys
import types

sys.path.insert(0, "/opt/trn_rl_repo")

import numpy as np


def _install_ntff_hook():
    """Recreate the missing antenv.axon_hooks module so trace=True works."""
    try:
        import antenv

        if "antenv.axon_hooks" in sys.modules:
            return
        m = types.ModuleType("antenv.axon_hooks")
        m._hook = None

        def set_axon_ntff_profile_hook(h):
            m._hook = h

        def get_axon_ntff_profile_hook():
            return m._hook

        m.set_axon_ntff_profile_hook = set_axon_ntff_profile_hook
        m.get_axon_ntff_profile_hook = get_axon_ntff_profile_hook
        sys.modules["antenv.axon_hooks"] = m
        antenv.axon_hooks = m
        from trn_agent_boot.trn_boot import _ntff_profile_via_ctypes

        so = "/opt/axon/libaxon_pjrt.so"
        if os.path.exists(so):
            set_axon_ntff_profile_hook(_ntff_profile_via_ctypes(so))
    except Exception:
        pass


_install_ntff_hook()

import ml_dtypes
import concourse.bass as bass
import concourse.tile as tile
from concourse import bacc, mybir
from concourse.bass_utils import run_bass_kernel_spmd

BF16 = mybir.dt.bfloat16
F32 = mybir.dt.float32

B, S, D = 2, 2048, 2048
QH, H = 16, 128          # heads, head dim
N_CORES = 8
GROUPS = 4               # tensor-parallel groups per batch
HPC = QH // GROUPS       # heads per core = 4
DQC = HPC * H            # per-core projection width = 512
NT = S // 128            # 16 s/d tiles of 128
NC = S // 512            # 4 chunks of 512
KT = D // 128            # 16 contraction tiles

LAST_RESULTS = None      # test harness reads exec_time_ns from here


def _build():
    nc = bacc.Bacc("TRN2", target_bir_lowering=False, debug=False,
                   num_devices=N_CORES)

    xt = [nc.dram_tensor(f"xt{sc}", [128, KT, 512], BF16,
                         kind="ExternalInput") for sc in range(NC)]
    wq = nc.dram_tensor("wq", [128, KT, DQC], BF16, kind="ExternalInput")
    wk = nc.dram_tensor("wk", [128, KT, DQC], BF16, kind="ExternalInput")
    wv = nc.dram_tensor("wv", [128, KT, DQC], BF16, kind="ExternalInput")
    wo = nc.dram_tensor("wo", [128, KT, DQC], BF16, kind="ExternalInput")
    cosT = nc.dram_tensor("cosT", [H, S], BF16, kind="ExternalInput")
    sinTs = nc.dram_tensor("sinTs", [H, S], BF16, kind="ExternalInput")
    tri = nc.dram_tensor("tri", [128, 128], BF16, kind="ExternalInput")
    out = nc.dram_tensor("out", [S, DQC], F32, kind="ExternalOutput")

    with tile.TileContext(nc) as tc:
        with (
            tc.tile_pool(name="big", bufs=4) as big_pool,      # xt / ytf pieces
            tc.tile_pool(name="wts", bufs=1) as wts_pool,      # weights
            tc.tile_pool(name="qkv", bufs=1) as qkv_pool,      # qt/kt/v/subs
            tc.tile_pool(name="small", bufs=1) as small_pool,  # constants
            tc.tile_pool(name="work", bufs=3) as work_pool,    # rope/at/tmp
            tc.tile_pool(name="psum", bufs=2, space="PSUM") as psum_pool,
            tc.tile_pool(name="dram", bufs=1, space="DRAM") as dram_pool,
        ):
            # ---- batched input DMAs, ordered for earliest PE start --------
            wq_sb = wts_pool.tile([128, KT, DQC], BF16, tag="wq", name="wq")
            nc.sync.dma_start(wq_sb[:], wq[:])
            xt_sb = []
            for sc in range(NC):
                t = big_pool.tile([128, KT, 512], BF16, tag="big",
                                  name=f"xt{sc}")
                xt_sb.append(t)
            nc.sync.dma_start(xt_sb[0][:], xt[0][:])
            wk_sb = wts_pool.tile([128, KT, DQC], BF16, tag="wk", name="wk")
            nc.sync.dma_start(wk_sb[:], wk[:])
            nc.sync.dma_start(xt_sb[1][:], xt[1][:])
            wv_sb = wts_pool.tile([128, KT, DQC], BF16, tag="wvo", name="wv")
            nc.sync.dma_start(wv_sb[:], wv[:])
            nc.sync.dma_start(xt_sb[2][:], xt[2][:])
            nc.sync.dma_start(xt_sb[3][:], xt[3][:])
            cos_sb = small_pool.tile([H, S], BF16, tag="cos", name="cos")
            nc.sync.dma_start(cos_sb[:], cosT[:])
            sin_sb = small_pool.tile([H, S], BF16, tag="sin", name="sin")
            nc.sync.dma_start(sin_sb[:], sinTs[:])
            tri_sb = small_pool.tile([128, 128], BF16, tag="tri", name="tri")
            nc.sync.dma_start(tri_sb[:], tri[:])
            ones_sb = small_pool.tile([128, 1], BF16, tag="ones", name="ones")
            nc.vector.memset(ones_sb[:], 1.0)
            onesr_sb = small_pool.tile([1, 128], BF16, tag="onesr",
                                       name="onesr")
            nc.vector.memset(onesr_sb[:], 1.0)

            qt_sb = [qkv_pool.tile([H, S], BF16, tag=f"qt{m}", name=f"qt{m}")
                     for m in range(HPC)]
            kt_sb = [qkv_pool.tile([H, S], BF16, tag=f"kt{m}", name=f"kt{m}")
                     for m in range(HPC)]
            v_sb = [qkv_pool.tile([128, DQC], BF16, tag=f"v{i}", name=f"v{i}")
                    for i in range(NT)]

            rope_n = [0]

            def rope(ps, dst, sc):
                """dst[:, sc-chunk] = ps*cos + swap_halves(ps)*sin_signed."""
                scol = slice(sc * 512, (sc + 1) * 512)
                i = rope_n[0]
                rope_n[0] += 1
                t_rot = work_pool.tile([128, 512], BF16, tag="t_rot",
                                       bufs=2, name=f"t_rot{i}")
                nc.vector.tensor_copy(t_rot[0:64, :], ps[64:128, :])
                nc.vector.tensor_copy(t_rot[64:128, :], ps[0:64, :])
                t_cos = work_pool.tile([128, 512], BF16, tag="t_cos",
                                       bufs=2, name=f"t_cos{i}")
                nc.vector.tensor_tensor(t_cos[:], ps[:], cos_sb[:, scol],
                                        mybir.AluOpType.mult)
                t_sin = work_pool.tile([128, 512], BF16, tag="t_sin",
                                       bufs=2, name=f"t_sin{i}")
                nc.vector.tensor_tensor(t_sin[:], t_rot[:], sin_sb[:, scol],
                                        mybir.AluOpType.mult)
                nc.vector.tensor_tensor(dst[:, scol], t_cos[:], t_sin[:],
                                        mybir.AluOpType.add)

            def proj_chain(m, w_sb, dst, sc, nm):
                """One 16-matmul projection chain + RoPE for (head m, chunk sc)."""
                ps = psum_pool.tile([128, 512], F32, tag="pp", bufs=2,
                                    name=f"pp_{nm}{m}{sc}")
                for kd in range(KT):
                    nc.tensor.matmul(
                        ps[:],
                        w_sb[:, kd, m * 128:(m + 1) * 128],
                        xt_sb[sc][:, kd, :],
                        start=(kd == 0), stop=(kd == KT - 1),
                    )
                rope(ps, dst, sc)

            def proj_thunks(m):
                """Per-matmul thunks for head m's 8 proj chains; each thunk
                returns True iff it emitted a PE matmul."""
                thunks = []
                for (w_sb, dst, nm) in ((wq_sb, qt_sb[m], "q"),
                                        (wk_sb, kt_sb[m], "k")):
                    for sc in range(NC):
                        hold = [None]

                        def mk_mm(kd, w_sb=w_sb, sc=sc, hold=hold, m=m, nm=nm):
                            def f():
                                if kd == 0:
                                    hold[0] = psum_pool.tile(
                                        [128, 512], F32, tag="pp", bufs=2,
                                        name=f"pp_{nm}{m}{sc}")
                                nc.tensor.matmul(
                                    hold[0][:],
                                    w_sb[:, kd, m * 128:(m + 1) * 128],
                                    xt_sb[sc][:, kd, :],
                                    start=(kd == 0), stop=(kd == KT - 1),
                                )
                                return True
                            return f

                        for kd in range(KT):
                            thunks.append(mk_mm(kd))

                        def mk_rope(dst=dst, sc=sc, hold=hold):
                            def f():
                                rope(hold[0], dst, sc)
                                return False
                            return f

                        thunks.append(mk_rope())
                return thunks

            def pop_filler(fillers):
                """Emit filler thunks until one PE matmul lands (or drained)."""
                while True:
                    th = next(fillers, None)
                    if th is None or th():
                        return

            def v_chain(i):
                """V projection for s-tile i."""
                sc, o = i // 4, (i % 4) * 128
                ps = psum_pool.tile([128, DQC], F32, tag="pp", bufs=2,
                                    name=f"pp_v{i}")
                for kd in range(KT):
                    nc.tensor.matmul(
                        ps[:],
                        xt_sb[sc][:, kd, o:o + 128],
                        wv_sb[:, kd, :],
                        start=(kd == 0), stop=(kd == KT - 1),
                    )
                nc.scalar.copy(v_sb[i][:], ps[:])

            # dram staging: one piece AllGather per head 0-2 (collectives
            # cost ~25us each regardless of size, so fewer is better), plus
            # per-q-chunk sub-gathers for head 3 that pipeline into stage F
            yt_piece = [dram_pool.tile([128, S], BF16, name=f"yt_p{m}")
                        for m in range(HPC - 1)]
            ytf_piece = [dram_pool.tile([DQC, S], BF16, name=f"ytf_p{m}")
                         for m in range(HPC - 1)]
            yt3_sub = [dram_pool.tile([128, 512], BF16, name=f"yt3_s{j}")
                       for j in range(NC)]
            ytf3_sub = [dram_pool.tile([DQC, 512], BF16, name=f"ytf3_s{j}")
                        for j in range(NC)]
            ytf_sb = [None] * (HPC - 1)   # heads 0-2 land in dead xt tiles
            ytf3sub_sb = [[None] * GROUPS for _ in range(NC)]

            # warmup collective: absorbs the expensive first-op setup (~40-
            # 100us observed) while the PE chews the prologue
            warm_in = dram_pool.tile([1, 64], BF16, name="warm_in")
            warm_out = dram_pool.tile([GROUPS, 64], BF16, name="warm_out")
            warm_sb = small_pool.tile([1, 64], BF16, tag="warm", name="warm")
            nc.vector.memset(warm_sb[:], 0.0)
            nc.sync.dma_start(warm_in[:], warm_sb[:])
            nc.gpsimd.collective_compute(
                "AllGather",
                mybir.AluOpType.bypass,
                replica_groups=[[0, 1, 2, 3], [4, 5, 6, 7]],
                ins=[warm_in.opt()],
                outs=[warm_out.opt()],
            )

            def load_piece(mh):
                """Load head mh's gathered piece into a dead xt tile."""
                t = big_pool.tile([128, KT, 512], BF16, tag="big",
                                  name=f"ytf{mh}")
                for p in range(GROUPS):
                    nc.sync.dma_start(
                        t[:, 4 * p:4 * p + 4, :],
                        ytf_piece[mh][p * 128:(p + 1) * 128, :])
                ytf_sb[mh] = t

            att_n = [0]

            def attn_head(m, chunks, fillers):
                """Causal attention for head m over the given q-chunks."""
                for j in chunks:
                    n_kv = 4 * j + 4
                    u = att_n[0]
                    att_n[0] += 1
                    ps_y = psum_pool.tile([128, 512], F32, tag="ps_y",
                                          bufs=2, name=f"ps_y{u}")
                    pden = psum_pool.tile([128, 512], F32, tag="pden",
                                          bufs=1, name=f"pden{u}")
                    state = {"pend": None, "accD": None, "pacc": []}

                    def kv_scores(t, u=u, m=m, j=j):
                        c0 = max(0, (t - 4 * j) * 128)
                        ps_s = psum_pool.tile([128, 512], F32, tag="ps_s",
                                              bufs=3, name=f"ps_s{u}_{t}")
                        nc.tensor.matmul(
                            ps_s[:, c0:512],
                            kt_sb[m][:, t * 128:(t + 1) * 128],
                            qt_sb[m][:, j * 512 + c0:(j + 1) * 512],
                            start=True, stop=True,
                        )
                        at = work_pool.tile([128, 512], BF16, tag="at",
                                            bufs=4, name=f"at{u}_{t}")
                        nc.scalar.activation(
                            at[:, c0:512], ps_s[:, c0:512],
                            mybir.ActivationFunctionType.Exp)
                        if t >= 4 * j:
                            nc.vector.tensor_tensor(
                                at[:, c0:c0 + 128], at[:, c0:c0 + 128],
                                tri_sb[:], mybir.AluOpType.mult)
                        return at, c0

                    def kv_accum(at, c0, t, u=u, m=m, j=j, n_kv=n_kv,
                                 ps_y=ps_y, state=state, fillers=fillers):
                        nc.tensor.matmul(
                            ps_y[:, c0:512],
                            v_sb[t][:, m * 128:(m + 1) * 128],
                            at[:, c0:512],
                            start=(t == 0), stop=(t == n_kv - 1),
                        )
                        pop_filler(fillers)
                        # denominator accumulation on the DVE
                        if t < 4 * j:
                            if state["pend"] is None:
                                state["pend"] = at
                            else:
                                p = work_pool.tile([128, 512], BF16,
                                                   tag="pacc", bufs=6,
                                                   name=f"pacc{u}_{t}")
                                nc.vector.tensor_tensor(
                                    p[:], state["pend"][:], at[:],
                                    mybir.AluOpType.add)
                                state["pacc"].append(p)
                                state["pend"] = None
                        else:
                            if state["accD"] is None:
                                state["accD"] = at       # t == 4j: c0 == 0
                            else:
                                nc.vector.tensor_tensor(
                                    state["accD"][:, c0:512],
                                    state["accD"][:, c0:512],
                                    at[:, c0:512], mybir.AluOpType.add)

                    # software pipeline: scores(t+1) issues before AV(t)
                    prev_at, prev_c0 = kv_scores(0)
                    for t in range(1, n_kv):
                        cur = kv_scores(t)
                        kv_accum(prev_at, prev_c0, t - 1)
                        prev_at, prev_c0 = cur
                    kv_accum(prev_at, prev_c0, n_kv - 1)

                    accD = state["accD"]
                    for p in state["pacc"]:
                        nc.vector.tensor_tensor(accD[:], accD[:], p[:],
                                                mybir.AluOpType.add)
                    # denominator row, reciprocal, then broadcast it back
                    # across partitions with a K=1 matmul (keeps the gpsimd
                    # queue free for collective dispatches)
                    nc.tensor.matmul(pden[0:1, :], ones_sb[:], accD[:],
                                     start=True, stop=True)
                    d_sb = work_pool.tile([1, 512], F32, tag="d_sb", bufs=2,
                                          name=f"d_sb{u}")
                    d_scr = work_pool.tile([1, 512], F32, tag="d_scr", bufs=2,
                                           name=f"d_scr{u}")
                    nc.vector.reciprocal_approx_accurate(
                        d_sb[:], pden[0:1, :], d_scr[:])
                    d_bf = work_pool.tile([1, 512], BF16, tag="d_bf", bufs=2,
                                          name=f"d_bf{u}")
                    nc.vector.tensor_copy(d_bf[:], d_sb[:])
                    nc.tensor.matmul(pden[:, :], onesr_sb[:], d_bf[:],
                                     start=True, stop=True)
                    b_sb = work_pool.tile([128, 512], BF16, tag="b_sb",
                                          bufs=2, name=f"b_sb{u}")
                    nc.vector.tensor_copy(b_sb[:], pden[:, :])
                    ytile = work_pool.tile([128, 512], BF16, tag="ytile",
                                           bufs=3, name=f"ytile{u}")
                    nc.vector.tensor_tensor(
                        ytile[:], ps_y[:], b_sb[:], mybir.AluOpType.mult)
                    if m == HPC - 1:
                        nc.sync.dma_start(yt3_sub[j][:], ytile[:])
                        nc.gpsimd.collective_compute(
                            "AllGather",
                            mybir.AluOpType.bypass,
                            replica_groups=[[0, 1, 2, 3], [4, 5, 6, 7]],
                            ins=[yt3_sub[j].opt()],
                            outs=[ytf3_sub[j].opt()],
                        )
                        if j < HPC - 2:
                            load_piece(j)
                    else:
                        nc.sync.dma_start(
                            yt_piece[m][:, j * 512:(j + 1) * 512], ytile[:])

            NO_FILL = iter(())

            # ---- prologue rounds, ordered to match DMA arrivals -----------
            proj_chain(0, wq_sb, qt_sb[0], 0, "q")
            proj_chain(0, wk_sb, kt_sb[0], 0, "k")
            proj_chain(0, wq_sb, qt_sb[0], 1, "q")
            proj_chain(0, wk_sb, kt_sb[0], 1, "k")
            for i in range(4):
                v_chain(i)
            attn_head(0, [0], NO_FILL)
            for i in range(4, 8):
                v_chain(i)
            proj_chain(0, wq_sb, qt_sb[0], 2, "q")
            proj_chain(0, wk_sb, kt_sb[0], 2, "k")
            for i in range(8, 12):
                v_chain(i)
            attn_head(0, [1], NO_FILL)
            proj_chain(0, wq_sb, qt_sb[0], 3, "q")
            proj_chain(0, wk_sb, kt_sb[0], 3, "k")
            for i in range(12, 16):
                v_chain(i)
            attn_head(0, [2], NO_FILL)
            # wo reuses wv's SBUF slot once the V chains drain
            wo_sb = wts_pool.tile([128, KT, DQC], BF16, tag="wvo", name="wo")
            nc.sync.dma_start(wo_sb[:], wo[:])

            # ---- attention train with woven next-head projections ---------
            def ag_piece(m):
                nc.gpsimd.collective_compute(
                    "AllGather",
                    mybir.AluOpType.bypass,
                    replica_groups=[[0, 1, 2, 3], [4, 5, 6, 7]],
                    ins=[yt_piece[m].opt()],
                    outs=[ytf_piece[m].opt()],
                )

            fillers = iter(proj_thunks(1))
            attn_head(0, [NC - 1], fillers)
            ag_piece(0)
            for th in fillers:
                th()
            for m in range(1, HPC):
                fillers = iter(proj_thunks(m + 1)) if m < HPC - 1 else NO_FILL
                attn_head(m, list(range(NC)), fillers)
                if m < HPC - 1:
                    ag_piece(m)
                for th in fillers:
                    th()

            load_piece(2)
            # head-3 gathered sub-pieces land in dead v slots
            for j in range(NC):
                for p in range(GROUPS):
                    t = qkv_pool.tile([128, 512], BF16, tag=f"v{4 * j + p}",
                                      name=f"y3s{j}_{p}")
                    nc.sync.dma_start(
                        t[:], ytf3_sub[j][p * 128:(p + 1) * 128, :])
                    ytf3sub_sb[j][p] = t

            # ---- stage F: output projection (512-col slice) ---------------
            for ms in range(NT):
                ps = psum_pool.tile([128, DQC], F32, tag="pp", bufs=2,
                                    name=f"pp_o{ms}")
                for mh in range(HPC):
                    for p in range(GROUPS):
                        if mh == HPC - 1:
                            lhsT = ytf3sub_sb[ms // 4][p][
                                :, (ms % 4) * 128:(ms % 4 + 1) * 128]
                        else:
                            lhsT = ytf_sb[mh][
                                :, 4 * p + ms // 4,
                                (ms % 4) * 128:(ms % 4) * 128 + 128]
                        nc.tensor.matmul(
                            ps[:],
                            lhsT,
                            wo_sb[:, 4 * p + mh, :],
                            start=(mh == 0 and p == 0),
                            stop=(mh == HPC - 1 and p == GROUPS - 1),
                        )
                o_sb = work_pool.tile([128, DQC], F32, tag="o_sb", bufs=2,
                                      name=f"o_sb{ms}")
                nc.scalar.copy(o_sb[:], ps[:])
                nc.sync.dma_start(out[ms * 128:(ms + 1) * 128, :], o_sb[:])

    nc.compile()
    return nc


_NC_CACHE = None


def kernel(x, wq, wk, wv, wo, mask, sin, cos):
    global LAST_RESULTS, _NC_CACHE
    bf16 = ml_dtypes.bfloat16

    def w_relayout(w):
        # [D, DQC] -> [128, KT, DQC]: partition p holds rows {kd*128+p}
        return np.ascontiguousarray(
            w.reshape(KT, 128, -1).transpose(1, 0, 2)).astype(bf16)

    wq_b = wq.astype(np.float32)
    wk_b = wk.astype(np.float32) * (H ** -0.5)   # fold k scaling into wk
    wv_b = wv.astype(np.float32)
    wo_b = wo.astype(np.float32)

    # transposed rope tables; sign-folded sin handles rotate_half:
    #   q'[0:64]   = q[0:64]*cos[0:64]   + q[64:128]*(-sin[0:64])
    #   q'[64:128] = q[64:128]*cos[64:]  + q[0:64]  *(+sin[64:])
    cosT = np.ascontiguousarray(cos.T).astype(bf16)                  # [H, S]
    sinT = np.ascontiguousarray(sin.T).astype(np.float32)
    sinTs = sinT.copy()
    sinTs[0:H // 2, :] *= -1.0
    sinTs = sinTs.astype(bf16)

    # multiplicative causal mask for the 128x128 diagonal blocks, in
    # [kv, q] orientation, derived from the additive mask input
    tri = (mask[:128, :128].T == 0.0).astype(bf16)

    if _NC_CACHE is None:
        _NC_CACHE = _build()
    nc = _NC_CACHE

    in_maps = []
    for c in range(N_CORES):
        b, g = c // GROUPS, c % GROUPS
        cols = slice(g * DQC, (g + 1) * DQC)
        # x[b].T -> [D, S] -> [NC, 128, KT, 512] column chunks
        xt = np.ascontiguousarray(x[b].T.astype(np.float32))
        xt4 = np.ascontiguousarray(
            xt.reshape(KT, 128, NC, 512).transpose(2, 1, 0, 3)).astype(bf16)
        in_maps.append({
            "xt0": np.ascontiguousarray(xt4[0]),
            "xt1": np.ascontiguousarray(xt4[1]),
            "xt2": np.ascontiguousarray(xt4[2]),
            "xt3": np.ascontiguousarray(xt4[3]),
            "wq": w_relayout(wq_b[:, cols]),
            "wk": w_relayout(wk_b[:, cols]),
            "wv": w_relayout(wv_b[:, cols]),
            "wo": w_relayout(wo_b[:, cols]),
            "cosT": cosT,
            "sinTs": sinTs,
            "tri": tri,
        })

    try:
        res = run_bass_kernel_spmd(nc, in_maps, core_ids=list(range(N_CORES)))
    except Exception:
        # transient device states (e.g. a prior crashed load) sometimes
        # surface as unrecoverable-execution errors; one retry clears them
        import time

        time.sleep(5)
        res = run_bass_kernel_spmd(nc, in_maps, core_ids=list(range(N_CORES)))
    LAST_RESULTS = res

    output = np.empty((B, S, D), dtype=np.float32)
    for c in range(N_CORES):
        b, g = c // GROUPS, c % GROUPS
        output[b, :, g * DQC:(g + 1) * DQC] = res.results[c]["out"]
    return output
